# revision 17
# baseline (speedup 1.0000x reference)
"""Trainium2 Bass kernel for nn_LowRankDynamicConv.

Math (per sample b):
  combined = [phrase_slot[b] | eos]                       [N, 2C]
  h        = relu(combined @ W1 + b1)                     [N, 4C]
  proj     = (h @ W2 + b2) viewed as [N*C, R]             [4096, 32]
  y        = x[b] @ proj   with x[b] = context_emb[b] as  [T, N*C]
  out[t]   = relu(LN(sum_{k,j} y[t + j - pad_k] @ M_kj + bo))
  where M_kj[r, co] = sum_d kjoin[kj, r, d] * Wo[k_block*C + d, co]

Key perf structure (v2):
  - x ships as fp8 e3m4 (scaled x2 on host; W2/b2 pre-halved to compensate),
    streamed straight into the stage-3 matmul as the moving operand against
    bf16 proj weights (mixed-dtype matmul).  Halves the biggest HBM stream.
  - All heavy DMAs use host-prepared layouts with per-partition-contiguous
    8-16KB runs (128 descriptors per DMA) - HWDGE descriptor generation at
    ~2.4ns/desc was the old bottleneck (1024x2KB descs per DMA).
  - The two skinny (M=32) matmul stages use 4-way PE column tiling
    (tile_position=(0,32j)): four concurrent M=32 matmuls in separate
    32-column groups of the PE array, each draining to its own PSUM bank
    partition slice.
  - W2 streams strictly before x (all 8 chunk DMAs dep-free up front) so
    proj is ready the moment the x stream begins; stage 3 chases x chunks.
  - y lives in a [128, 516] SBUF tile: partition group 32*(2b+hf) holds
    sample b's T-half hf, so every PSUM evacuation is lane-aligned; the
    conv-tap shifted copies (yk tiles) are small SBUF->SBUF DMAs.
  - LayerNorm pipeline is spread across ACT (normalize via per-partition
    scale/bias), DVE (stats, relu) and GpSimd (gamma/beta) so no single
    engine paces the emit tail.
"""
import sys

sys.path.insert(0, "/opt/trn_rl_repo")

import ml_dtypes
import numpy as np

import concourse.bass as bass  # noqa: F401  (bass types used via bacc)
import concourse.mybir as mybir
import concourse.tile as tile
from concourse import bacc
from concourse.bass_utils import run_bass_kernel_spmd
from concourse.masks import make_identity

F32 = mybir.dt.float32
BF16 = mybir.dt.bfloat16
FP8 = mybir.dt.float8e3
RELU = mybir.ActivationFunctionType.Relu
SQRT = mybir.ActivationFunctionType.Sqrt
IDENT = mybir.ActivationFunctionType.Identity
BF = ml_dtypes.bfloat16
E3 = ml_dtypes.float8_e3m4

X_FP8 = True               # ship x as fp8 e3m4 (x2, W2/b2 pre-halved)

NCORES = 8
BPC = 2                    # samples per core
T, N, C, R = 1024, 16, 256, 32
NCF = N * C                # 4096 flattened (n, c) contraction dim
CH = NCF // 128            # 32 nc-chunks of 128
GCH = 8                    # nc-chunks per x DMA tile
XT = CH // GCH             # x DMA tiles per sample (4)
QT = 256                   # T-quarter (stage-3 col-group free dim)
YCW = T + 4                # consolidated y width incl 2+2 pad columns
# (wo-block, temporal offset) per fused tap, in k1 | k3 | k5 order
JOFF = [(0, 0), (1, -1), (1, 0), (1, 1), (2, -2), (2, -1), (2, 0), (2, 1), (2, 2)]


def _broadcast_ap(ap, parts):
    """DMA access pattern replicating a 1D/2D DRAM tensor across `parts` partitions."""
    a = ap
    return bass.AP(tensor=a.tensor, offset=a.offset, ap=[[0, parts]] + list(a.ap))


def _build():
    nc = bacc.Bacc("TRN2", num_devices=NCORES)

    xq = nc.dram_tensor("xq", [BPC, 128, CH, T], FP8 if X_FP8 else BF16,
                        kind="ExternalInput")
    # small inputs ride in three packed blobs so startup is 4 DMAs, not 9:
    # blobA bf16 [128, 5634] = W1 (ko,m | 4096) | Wo (fc,co | 1536) | eos (2)
    # blobB bf16 [32, 2560]  = phrase (256) | kjoin taps (r-major | 2304)
    # blobC f32  [128, 776]  = b1 (mo | 8) | gamma|beta|bo replicated (768)
    blobAd = nc.dram_tensor("blobA", [128, 5634], BF16, kind="ExternalInput")
    blobBd = nc.dram_tensor("blobB", [BPC * N, 2560], BF16, kind="ExternalInput")
    blobCd = nc.dram_tensor("blobC", [128, 776], F32, kind="ExternalInput")
    w2h = nc.dram_tensor("w2h", [128, 8, 8, 1024], BF16, kind="ExternalInput")
    b2 = nc.dram_tensor("b2", [C * R], F32, kind="ExternalInput")
    out = nc.dram_tensor("out", [BPC, 2, 128, 4 * C], BF16,
                         kind="ExternalOutput")

    with tile.TileContext(nc) as tc:
        with tc.tile_pool(name="keep", bufs=1) as keep, \
             tc.tile_pool(name="pXg", bufs=4) as pXg, \
             tc.tile_pool(name="dram", bufs=1, space="DRAM") as dram:
            ident = keep.tile([128, 128], BF16)
            make_identity(nc, ident)

            # all four startup loads on the sync ring AHEAD of the W2 chunks
            blobC = keep.tile([128, 776], F32)
            nc.sync.dma_start(blobC, blobCd[:, :])
            gsb = blobC[:, 8:8 + C]
            bsb = blobC[:, 8 + C:8 + 2 * C]
            bosb = blobC[:, 8 + 2 * C:8 + 3 * C]
            epsb = keep.tile([128, 1], F32)
            nc.vector.memset(epsb, 1e-5)
            # b2 regrouped to the post-reshard proj layout: [c%128, c-half, r]
            b2v = keep.tile([128, 2, R], F32)
            nc.sync.dma_start(b2v, b2[:].rearrange("(c2 p r) -> p c2 r", p=128, r=R))
            # bf16 gamma/beta for the elementwise post-normalize ops
            gb16 = keep.tile([128, 1, C], BF16)
            nc.vector.tensor_copy(gb16[:, 0, :], gsb)
            bb16 = keep.tile([128, 1, C], BF16)
            nc.vector.tensor_copy(bb16[:, 0, :], bsb)

            # stage-3 lhsT tiles [c%128 part, (b, c-half, n), r] bf16
            projf = keep.tile([128, BPC * CH, R], BF16)
            # fused conv+output weights M_kj [r part, tap, co], plus stacked
            # rhs tiles for the 3-matmul emit: taps 0-3 | taps 4-7 | tap 8+bo
            msb = keep.tile([R, 9, C], BF16)
            mst0 = keep.tile([128, C], BF16)
            mst1 = keep.tile([128, C], BF16)
            mcat = keep.tile([R + 1, C], BF16)
            # h^T persists through phase A
            hT = keep.tile([128, 8, BPC * N], BF16)

            # ---- phase A -------------------------------------------------------
            with tc.tile_pool(name="pA", bufs=1) as pA, \
                 tc.tile_pool(name="pW2", bufs=8) as pW2, \
                 tc.tile_pool(name="pAs", bufs=2) as pAs, \
                 tc.tile_pool(name="psA", bufs=2, space="PSUM") as psA, \
                 tc.tile_pool(name="psW", bufs=4, space="PSUM") as psW:
                blobA = pA.tile([128, 5634], BF16)
                nc.sync.dma_start(blobA, blobAd[:, :])
                blobB = pA.tile([BPC * N, 2560], BF16)
                nc.sync.dma_start(blobB, blobBd[:, :])

                def w1s(ko, lo, hi):
                    return blobA[:, ko * 1024 + lo:ko * 1024 + hi]

                def wos(fc):
                    return blobA[:, 4096 + fc * C:4096 + (fc + 1) * C]

                def eoss(o):
                    return blobA[:, 5632 + o:5633 + o]

                def phs(lo, hi):
                    return blobB[:, lo:hi]

                def kjs(ji, dc):
                    return blobB[:, 256 + ji * C + dc * 128:
                                  256 + ji * C + (dc + 1) * 128]

                w2sb = []
                for j8 in range(8):
                    w = pW2.tile([128, 8, 1024], BF16, tag="w2", name=f"w2_{j8}")
                    nc.sync.dma_start(w, w2h[:, j8, :, :])
                    w2sb.append(w)

                # combined^T [c2%128 part, ko, bn] bf16
                combT = pA.tile([128, 4, BPC * N], BF16)
                for ko in range(2):
                    pt = psA.tile([128, BPC * N], BF16, tag="t")
                    nc.tensor.transpose(pt, phs(ko * 128, (ko + 1) * 128),
                                        ident[:BPC * N, :BPC * N])
                    nc.vector.tensor_copy(combT[:, ko, :], pt)
                for o in range(2):
                    nc.vector.tensor_copy(
                        combT[:, 2 + o, :],
                        eoss(o).to_broadcast((128, BPC * N)))

                # h^T [m%128 part, mo, bn] = relu(W1^T combined + b1), bf16
                for mo in range(8):
                    ph = psA.tile([128, BPC * N], F32, tag="t")
                    for ko in range(4):
                        nc.tensor.matmul(ph, w1s(ko, mo * 128, (mo + 1) * 128),
                                         combT[:, ko, :],
                                         start=(ko == 0), stop=(ko == 3))
                    nc.scalar.activation(out=hT[:, mo, :], in_=ph, func=RELU,
                                         bias=blobC[:, mo:mo + 1], scale=1.0)

                # M_kj = kjoin_kj @ Wo_block: transpose kjoin taps, then contract
                kjT = pA.tile([128, 2 * 9, R], BF16)
                for ji in range(9):
                    for dc in range(2):
                        pt = psA.tile([128, R], BF16, tag="t")
                        nc.tensor.transpose(pt, kjs(ji, dc),
                                            ident[:R, :R])
                        nc.vector.tensor_copy(kjT[:, ji * 2 + dc, :], pt)
                for ji, (kb, _off) in enumerate(JOFF):
                    pm = psA.tile([R, C], F32, tag="t")
                    for dc in range(2):
                        nc.tensor.matmul(pm, kjT[:, ji * 2 + dc, :],
                                         wos(kb * 2 + dc),
                                         start=(dc == 0), stop=(dc == 1))
                    nc.vector.tensor_copy(msb[:, ji, :], pm)
                # stack the emit rhs: taps 0-3 / 4-7 on 128 partitions; tap 8
                # plus the bo row on 33 (gpsimd queue: tiny, off the big FIFO)
                for q in range(4):
                    nc.gpsimd.dma_start(mst0[q * R:(q + 1) * R, :], msb[:, q, :])
                    nc.gpsimd.dma_start(mst1[q * R:(q + 1) * R, :], msb[:, 4 + q, :])
                nc.gpsimd.dma_start(mcat[0:R, :], msb[:, 8, :])
                nc.vector.tensor_copy(mcat[R:R + 1, :], bosb[0:1, :])

                # proj rows via 4-way column-tiled matmuls: set s covers
                # m-cols [s*2048, (s+1)*2048); group j streams its own 512
                # W2 columns into PE column group j concurrently (M=32 each)
                scratch = dram.tile([BPC * N, C * R], BF16)

                def reshard(c2):
                    # scratch cols [c2*4096, +4096) are final once sets
                    # 2*c2, 2*c2+1 have written; reshard them immediately so
                    # stage 3 is not gated on the whole of phase A
                    for b in range(BPC):
                        dst = projf[:, b * CH + c2 * N:b * CH + c2 * N + N, :]
                        nc.gpsimd.dma_start(
                            dst,
                            scratch[b * N:(b + 1) * N, c2 * 4096:(c2 + 1) * 4096]
                            .rearrange("n (p r) -> p n r", p=128, r=R))
                        nc.vector.tensor_add(
                            dst, dst,
                            b2v[:, c2:c2 + 1, :].to_broadcast((128, N, R)))

                for s in range(4):
                    psum = [psW.tile([128, 512], F32, tag="pj", name=f"pj{s}_{j}")
                            for j in range(4)]
                    for ko in range(8):
                        for j in range(4):
                            wch = w2sb[2 * s + j // 2]
                            q2 = j % 2
                            nc.tensor.matmul(
                                psum[j][32 * j:32 * j + 32, :],
                                hT[:, ko, :],
                                wch[:, ko, q2 * 512:(q2 + 1) * 512],
                                start=(ko == 0), stop=(ko == 7),
                                tile_position=(0, 32 * j))
                    pjsb = pAs.tile([128, 512], BF16, tag="pjsb")
                    for j in range(4):
                        nc.vector.tensor_copy(pjsb[32 * j:32 * j + 32, :],
                                              psum[j][32 * j:32 * j + 32, :])
                        nc.scalar.dma_start(
                            scratch[:, s * 2048 + j * 512:s * 2048 + (j + 1) * 512],
                            pjsb[32 * j:32 * j + 32, :])
                    if s == 1:
                        reshard(0)
                    elif s == 3:
                        reshard(1)

            # ---- phase X: streamed x tiles, col-tiled stage 3, emit -----------
            xgs = {}
            for b in range(BPC):
                for g in range(XT):
                    xg = pXg.tile([128, GCH, T], FP8 if X_FP8 else BF16,
                                  tag="xg", name=f"xg{b}_{g}")
                    nc.sync.dma_start(xg, xq[b, :, g * GCH:(g + 1) * GCH, :])
                    xgs[(b, g)] = xg

            with tc.tile_pool(name="pXw", bufs=6) as pXw, \
                 tc.tile_pool(name="pY", bufs=2) as pY, \
                 tc.tile_pool(name="obuf4", bufs=4) as obuf4, \
                 tc.tile_pool(name="yp", bufs=4, space="PSUM") as yp, \
                 tc.tile_pool(name="op", bufs=4, space="PSUM") as op:

                def s3(b, pys):
                    # stage 3: the four T-quarters of sample b accumulate
                    # across the 32 nc-chunks in four concurrent PE column
                    # groups (M=32 each), one PSUM bank per quarter
                    for ch in range(CH):
                        lhs = projf[:, b * CH + (ch % 2) * N + ch // 2, :]
                        xg = xgs[(b, ch // GCH)]
                        for q in range(4):
                            nc.tensor.matmul(
                                pys[q][32 * q:32 * q + 32, :],
                                lhs,
                                xg[:, ch % GCH, q * QT:(q + 1) * QT],
                                start=(ch == 0), stop=(ch == CH - 1),
                                tile_position=(0, 32 * q))

                def yfin(b, pys):
                    # evacuate quarters (lane-aligned), consolidate into the
                    # contiguous padded ysbc (+ ones row for the bo matmul),
                    # then one shifted copy per conv tap
                    ysbq = pY.tile([128, QT], BF16, tag="ysbq", name=f"ysbq{b}")
                    for q in range(4):
                        nc.vector.tensor_copy(ysbq[32 * q:32 * q + 32, :],
                                              pys[q][32 * q:32 * q + 32, :])
                    ysbc = pY.tile([R + 1, YCW], BF16, tag="ysbc", name=f"ysbc{b}")
                    nc.vector.memset(ysbc[0:R, 0:2], 0.0)
                    nc.vector.memset(ysbc[0:R, YCW - 2:YCW], 0.0)
                    nc.gpsimd.memset(ysbc[R:R + 1, :], 1.0)
                    for q in range(4):
                        eng = nc.scalar if q % 2 == 0 else nc.gpsimd
                        eng.dma_start(ysbc[0:R, 2 + q * QT:2 + (q + 1) * QT],
                                      ysbq[32 * q:32 * q + 32, :])
                    yk0 = pY.tile([128, T], BF16, tag="yk0", name=f"yk0_{b}")
                    yk1 = pY.tile([128, T], BF16, tag="yk1", name=f"yk1_{b}")
                    for q, (_kb, off) in enumerate(JOFF[0:4]):
                        nc.scalar.dma_start(yk0[q * R:(q + 1) * R, :],
                                            ysbc[0:R, off + 2:off + 2 + T])
                    for q, (_kb, off) in enumerate(JOFF[4:8]):
                        nc.gpsimd.dma_start(yk1[q * R:(q + 1) * R, :],
                                            ysbc[0:R, off + 2:off + 2 + T])
                    return ysbc, yk0, yk1

                def emit(b, ysbc, yk0, yk1):
                    # emit: 3 stacked matmuls (tap 8 + bo ride directly on the
                    # ysbc rows incl. its ones row) + LN + relu per tile; the
                    # gamma/beta/relu passes run pair-merged on two tiles
                    zn2 = None
                    for ts in range(T // 128):
                        po = op.tile([128, C], F32, tag="o")
                        nc.tensor.matmul(po, yk0[:, ts * 128:(ts + 1) * 128],
                                         mst0, start=True, stop=False)
                        nc.tensor.matmul(po, yk1[:, ts * 128:(ts + 1) * 128],
                                         mst1, start=False, stop=False)
                        nc.tensor.matmul(po, ysbc[:, 4 + ts * 128:4 + (ts + 1) * 128],
                                         mcat, start=False, stop=True)
                        st = pXw.tile([128, 6], F32, tag="st")
                        nc.vector.bn_stats(out=st, in_=po)
                        mv = pXw.tile([128, 2], F32, tag="mv")
                        nc.vector.bn_aggr(out=mv, in_=st)
                        rs = pXw.tile([128, 1], F32, tag="rs")
                        nc.scalar.activation(out=rs, in_=mv[:, 1:2], func=SQRT,
                                             bias=epsb, scale=1.0)
                        nc.vector.reciprocal(rs, rs)
                        nmr = pXw.tile([128, 1], F32, tag="nmr")
                        nc.vector.tensor_scalar(nmr, mv[:, 0:1], rs[:, 0:1], -1.0,
                                                mybir.AluOpType.mult,
                                                mybir.AluOpType.mult)
                        if ts % 2 == 0:
                            zn2 = pXw.tile([128, 2, C], BF16, tag="zn2")
                        nc.scalar.activation(out=zn2[:, ts % 2, :], in_=po,
                                             func=IDENT,
                                             bias=nmr[:, 0:1], scale=rs[:, 0:1])
                        if ts % 4 == 0:
                            ob = obuf4.tile([128, 4, C], BF16, tag="ob4",
                                            name=f"ob{b}_{ts}")
                        if ts % 2 == 1:
                            zg2 = pXw.tile([128, 2, C], BF16, tag="zg2")
                            nc.gpsimd.tensor_mul(zg2, zn2,
                                                 gb16[:, 0:1, :].to_broadcast((128, 2, C)))
                            nc.gpsimd.tensor_add(zg2, zg2,
                                                 bb16[:, 0:1, :].to_broadcast((128, 2, C)))
                            h = (ts % 4) - 1
                            nc.vector.tensor_scalar_max(ob[:, h:h + 2, :], zg2, 0.0)
                        if ts % 4 == 3:
                            nc.scalar.dma_start(
                                out[b, ts // 4, :, :]
                                .rearrange("p (q c) -> p q c", q=4), ob)

                for b in range(BPC):
                    pys = [yp.tile([128, QT], F32, tag="y", name=f"py{b}_{q}")
                           for q in range(4)]
                    s3(b, pys)
                    ysbc, yk0, yk1 = yfin(b, pys)
                    emit(b, ysbc, yk0, yk1)

    nc.compile()
    return nc


_NC = None


def _get_nc():
    global _NC
    if _NC is None:
        _NC = _build()
    return _NC


def _shard(inputs):
    """Split full inputs into per-core input maps (layout/cast only)."""
    x = np.asarray(inputs["context_emb"], dtype=np.float32)
    B = x.shape[0]
    assert B == NCORES * BPC
    # [B, T, N, C] -> [B, NCF, T] -> [B, 128, CH, T], fp8 e3m4 at 2x scale
    xT = np.swapaxes(x.reshape(B, T, NCF), 1, 2)
    xT = np.ascontiguousarray(
        np.swapaxes(xT.reshape(B, CH, 128, T), 1, 2))
    if X_FP8:
        xq = np.clip(xT * 2.0, -15.0, 15.0).astype(E3)
        w2scale = 0.5
    else:
        xq = xT.astype(BF)
        w2scale = 1.0
    ph = np.asarray(inputs["phrase_slot"], dtype=np.float32)
    w2 = np.asarray(inputs["W2"], dtype=np.float32) * w2scale
    w2h = np.ascontiguousarray(
        w2.reshape(8, 128, 8, 1024).transpose(1, 2, 0, 3)).astype(BF)
    w1 = np.asarray(inputs["W1"], dtype=np.float32)
    w1h = w1.reshape(4, 128, 4 * C).transpose(1, 0, 2).reshape(128, 4096)
    kjoin = np.concatenate(
        [np.moveaxis(inputs[f"k{k}"], 2, 0) for k in (1, 3, 5)],
        axis=0)  # [9, 32, 256]
    kjh = np.moveaxis(kjoin, 1, 0).reshape(R, 9 * C)  # [r, (j d)]
    wo = np.asarray(inputs["Wo"], dtype=np.float32)
    woh = wo.reshape(6, 128, C).transpose(1, 0, 2).reshape(128, 6 * C)
    eos = np.asarray(inputs["eos_slot"], dtype=np.float32).reshape(2, 128).T
    blobA = np.ascontiguousarray(
        np.concatenate([w1h, woh, eos], axis=1)).astype(BF)
    b1c = np.asarray(inputs["b1"], dtype=np.float32).reshape(8, 128).T
    lnp = np.concatenate([
        np.asarray(inputs["gamma"], dtype=np.float32),
        np.asarray(inputs["beta"], dtype=np.float32),
        np.asarray(inputs["bo"], dtype=np.float32)])
    blobC = np.ascontiguousarray(np.concatenate(
        [b1c, np.broadcast_to(lnp, (128, 3 * C))], axis=1), dtype=np.float32)
    shared = {
        "blobA": blobA,
        "blobC": blobC,
        "w2h": w2h,
        "b2": np.ascontiguousarray(
            np.asarray(inputs["b2"], dtype=np.float32) * w2scale),
    }
    in_maps = []
    for i in range(NCORES):
        m = dict(shared)
        m["xq"] = np.ascontiguousarray(xq[i * BPC:(i + 1) * BPC])
        m["blobB"] = np.ascontiguousarray(np.concatenate(
            [ph[i * BPC:(i + 1) * BPC].reshape(BPC * N, C), kjh],
            axis=1)).astype(BF)
        in_maps.append(m)
    return in_maps


def _run(inputs, **kwargs):
    nc = _get_nc()
    res = run_bass_kernel_spmd(nc, _shard(inputs), core_ids=list(range(NCORES)),
                               **kwargs)
    outs = [r["out"] for r in res.results]
    full = np.concatenate(outs, axis=0).reshape(NCORES * BPC, 2, 128, 4, C)
    # [b, s, p, q, c] -> t = (s*4 + q)*128 + p
    full = np.ascontiguousarray(full.transpose(0, 1, 3, 2, 4)).reshape(
        NCORES * BPC, T, C)
    return full.astype(np.float32), res


def kernel(**inputs) -> np.ndarray:
    out, _ = _run(inputs)
    return out


# revision 27
# speedup vs baseline: 1.1485x; 1.1485x over previous
"""Trainium2 Bass kernel for nn_LowRankDynamicConv.

Math (per sample b):
  combined = [phrase_slot[b] | eos]                       [N, 2C]
  h        = relu(combined @ W1 + b1)                     [N, 4C]
  proj     = (h @ W2 + b2) viewed as [N*C, R]             [4096, 32]
  y        = x[b] @ proj   with x[b] = context_emb[b] as  [T, N*C]
  out[t]   = relu(LN(sum_{k,j} y[t + j - pad_k] @ M_kj + bo))
  where M_kj[r, co] = sum_d kjoin[kj, r, d] * Wo[k_block*C + d, co]

Key perf structure (v2):
  - x ships as fp8 e3m4 (scaled x2 on host; W2/b2 pre-halved to compensate),
    streamed straight into the stage-3 matmul as the moving operand against
    bf16 proj weights (mixed-dtype matmul).  Halves the biggest HBM stream.
  - All heavy DMAs use host-prepared layouts with per-partition-contiguous
    8-16KB runs (128 descriptors per DMA) - HWDGE descriptor generation at
    ~2.4ns/desc was the old bottleneck (1024x2KB descs per DMA).
  - The two skinny (M=32) matmul stages use 4-way PE column tiling
    (tile_position=(0,32j)): four concurrent M=32 matmuls in separate
    32-column groups of the PE array, each draining to its own PSUM bank
    partition slice.
  - W2 streams strictly before x (all 8 chunk DMAs dep-free up front) so
    proj is ready the moment the x stream begins; stage 3 chases x chunks.
  - y lives in a [128, 516] SBUF tile: partition group 32*(2b+hf) holds
    sample b's T-half hf, so every PSUM evacuation is lane-aligned; the
    conv-tap shifted copies (yk tiles) are small SBUF->SBUF DMAs.
  - LayerNorm pipeline is spread across ACT (normalize via per-partition
    scale/bias), DVE (stats, relu) and GpSimd (gamma/beta) so no single
    engine paces the emit tail.
"""
import sys

sys.path.insert(0, "/opt/trn_rl_repo")

import ml_dtypes
import numpy as np

import concourse.bass as bass  # noqa: F401  (bass types used via bacc)
import concourse.mybir as mybir
import concourse.tile as tile
from concourse import bacc
from concourse.bass_utils import run_bass_kernel_spmd
from concourse.masks import make_identity

F32 = mybir.dt.float32
BF16 = mybir.dt.bfloat16
FP8 = mybir.dt.float8e3
RELU = mybir.ActivationFunctionType.Relu
SQRT = mybir.ActivationFunctionType.Sqrt
IDENT = mybir.ActivationFunctionType.Identity
BF = ml_dtypes.bfloat16
E3 = ml_dtypes.float8_e3m4

X_FP8 = True               # ship x as fp8 e3m4 (x2, W2/b2 pre-halved)

NCORES = 8
BPC = 2                    # samples per core
T, N, C, R = 1024, 16, 256, 32
NCF = N * C                # 4096 flattened (n, c) contraction dim
CH = NCF // 128            # 32 nc-chunks of 128
GCH = 8                    # nc-chunks per x DMA tile
XT = CH // GCH             # x DMA tiles per sample (4)
QT = 256                   # T-quarter (stage-3 col-group free dim)
YCW = T + 4                # consolidated y width incl 2+2 pad columns
# (wo-block, temporal offset) per fused tap, in k1 | k3 | k5 order
JOFF = [(0, 0), (1, -1), (1, 0), (1, 1), (2, -2), (2, -1), (2, 0), (2, 1), (2, 2)]


def _broadcast_ap(ap, parts):
    """DMA access pattern replicating a 1D/2D DRAM tensor across `parts` partitions."""
    a = ap
    return bass.AP(tensor=a.tensor, offset=a.offset, ap=[[0, parts]] + list(a.ap))


def _build():
    nc = bacc.Bacc("TRN2", num_devices=NCORES)

    xq = nc.dram_tensor("xq", [BPC, 128, CH, T], FP8 if X_FP8 else BF16,
                        kind="ExternalInput")
    # small inputs ride in three packed blobs so startup is 3 DMAs, not 9:
    # blobA bf16 [128, 5634] = W1 (ko,m | 4096) | Wo (fc,co | 1536) | eos (2)
    # blobB bf16 [32, 2560]  = phrase (256) | kjoin taps (r-major | 2304)
    # blobC f32  [128, 2824] = b1 (mo | 8) | gamma|beta|bo replicated (768)
    #                          | b2 grouped per (set, col-group) (2048)
    blobAd = nc.dram_tensor("blobA", [128, 5634], BF16, kind="ExternalInput")
    blobBd = nc.dram_tensor("blobB", [BPC * N, 2560], BF16, kind="ExternalInput")
    blobCd = nc.dram_tensor("blobC", [128, 2824], F32, kind="ExternalInput")
    w2h = nc.dram_tensor("w2h", [128, 8, 8, 1024], BF16, kind="ExternalInput")
    out = nc.dram_tensor("out", [BPC, 2, 128, 4 * C], BF16,
                         kind="ExternalOutput")

    with tile.TileContext(nc) as tc:
        with tc.tile_pool(name="keep", bufs=1) as keep, \
             tc.tile_pool(name="pXg", bufs=4) as pXg, \
             tc.tile_pool(name="dram", bufs=1, space="DRAM") as dram:
            ident = keep.tile([128, 128], BF16)
            make_identity(nc, ident)

            # all startup loads on the sync ring AHEAD of the W2 chunks
            blobC = keep.tile([128, 2824], F32)
            nc.sync.dma_start(blobC, blobCd[:, :])
            gsb = blobC[:, 8:8 + C]
            bsb = blobC[:, 8 + C:8 + 2 * C]
            bosb = blobC[:, 8 + 2 * C:8 + 3 * C]
            epsb = keep.tile([128, 1], F32)
            nc.vector.memset(epsb, 1e-5)
            # bf16 gamma/beta for the elementwise post-normalize ops
            gb16 = keep.tile([128, 1, C], BF16)
            nc.vector.tensor_copy(gb16[:, 0, :], gsb)
            bb16 = keep.tile([128, 1, C], BF16)
            nc.vector.tensor_copy(bb16[:, 0, :], bsb)

            # consolidated y: rows [64b, 64b+32) = sample b's y (bf16, padded
            # 2 cols each side); rows 64b+32 = ones (for the bo contraction).
            # Ones + pads are set once here, long before first use.
            ysbc = keep.tile([128, YCW], BF16)
            for b in range(BPC):
                nc.vector.memset(ysbc[64 * b + R:64 * b + R + 1, :], 1.0)
                nc.vector.memset(ysbc[64 * b:64 * b + R, 0:2], 0.0)
                nc.vector.memset(ysbc[64 * b:64 * b + R, YCW - 2:YCW], 0.0)

            # stage-3 lhsT tiles [c%128 part, (b, c-half, n), r] bf16
            projf = keep.tile([128, BPC * CH, R], BF16)
            # fused conv+output weights M_kj [r part, tap, co], plus stacked
            # rhs tiles for the 3-matmul emit: taps 0-3 | taps 4-7 | tap 8+bo
            msb = keep.tile([R, 9, C], BF16)
            mst0 = keep.tile([128, C], BF16)
            mst1 = keep.tile([128, C], BF16)
            # mcat rows [0,33) and [64,97): same tap-8+bo data, replicated so
            # each sample's ysbc partition group has an aligned rhs
            mcat = keep.tile([128, C], BF16)
            # h^T persists through phase A
            hT = keep.tile([128, 8, BPC * N], BF16)

            # ---- phase A -------------------------------------------------------
            with tc.tile_pool(name="pA", bufs=1) as pA, \
                 tc.tile_pool(name="pW2", bufs=8) as pW2, \
                 tc.tile_pool(name="pAs", bufs=2) as pAs, \
                 tc.tile_pool(name="psA", bufs=2, space="PSUM") as psA, \
                 tc.tile_pool(name="psW", bufs=4, space="PSUM") as psW:
                blobA = pA.tile([128, 5634], BF16)
                nc.sync.dma_start(blobA, blobAd[:, :])
                blobB = pA.tile([BPC * N, 2560], BF16)
                nc.sync.dma_start(blobB, blobBd[:, :])

                def w1s(ko, lo, hi):
                    return blobA[:, ko * 1024 + lo:ko * 1024 + hi]

                def wos(fc):
                    return blobA[:, 4096 + fc * C:4096 + (fc + 1) * C]

                def eoss(o):
                    return blobA[:, 5632 + o:5633 + o]

                def phs(lo, hi):
                    return blobB[:, lo:hi]

                def kjs(ji, dc):
                    return blobB[:, 256 + ji * C + dc * 128:
                                  256 + ji * C + (dc + 1) * 128]

                w2sb = []
                for j8 in range(8):
                    w = pW2.tile([128, 8, 1024], BF16, tag="w2", name=f"w2_{j8}")
                    nc.sync.dma_start(w, w2h[:, j8, :, :])
                    w2sb.append(w)

                # combined^T [c2%128 part, ko, bn] bf16
                combT = pA.tile([128, 4, BPC * N], BF16)
                for ko in range(2):
                    pt = psA.tile([128, BPC * N], BF16, tag="t")
                    nc.tensor.transpose(pt, phs(ko * 128, (ko + 1) * 128),
                                        ident[:BPC * N, :BPC * N])
                    nc.vector.tensor_copy(combT[:, ko, :], pt)
                for o in range(2):
                    nc.vector.tensor_copy(
                        combT[:, 2 + o, :],
                        eoss(o).to_broadcast((128, BPC * N)))

                # h^T [m%128 part, mo, bn] = relu(W1^T combined + b1), bf16
                for mo in range(8):
                    ph = psA.tile([128, BPC * N], F32, tag="t")
                    for ko in range(4):
                        nc.tensor.matmul(ph, w1s(ko, mo * 128, (mo + 1) * 128),
                                         combT[:, ko, :],
                                         start=(ko == 0), stop=(ko == 3))
                    nc.scalar.activation(out=hT[:, mo, :], in_=ph, func=RELU,
                                         bias=blobC[:, mo:mo + 1], scale=1.0)

                # M_kj = kjoin_kj @ Wo_block: transpose kjoin taps, then contract
                kjT = pA.tile([128, 2 * 9, R], BF16)
                for ji in range(9):
                    for dc in range(2):
                        pt = psA.tile([128, R], BF16, tag="t")
                        nc.tensor.transpose(pt, kjs(ji, dc),
                                            ident[:R, :R])
                        nc.vector.tensor_copy(kjT[:, ji * 2 + dc, :], pt)
                for ji, (kb, _off) in enumerate(JOFF):
                    pm = psA.tile([R, C], F32, tag="t")
                    for dc in range(2):
                        nc.tensor.matmul(pm, kjT[:, ji * 2 + dc, :],
                                         wos(kb * 2 + dc),
                                         start=(dc == 0), stop=(dc == 1))
                    nc.vector.tensor_copy(msb[:, ji, :], pm)
                # stack the emit rhs: taps 0-3 / 4-7 on 128 partitions; tap 8
                # plus the bo row on 33 (gpsimd queue: tiny, off the big FIFO)
                for q in range(4):
                    nc.gpsimd.dma_start(mst0[q * R:(q + 1) * R, :], msb[:, q, :])
                    nc.gpsimd.dma_start(mst1[q * R:(q + 1) * R, :], msb[:, 4 + q, :])
                for b in range(BPC):
                    nc.gpsimd.dma_start(mcat[64 * b:64 * b + R, :], msb[:, 8, :])
                    nc.vector.tensor_copy(mcat[64 * b + R:64 * b + R + 1, :],
                                          bosb[0:1, :])

                # proj rows via 4-way column-tiled matmuls: set s covers
                # m-cols [s*2048, (s+1)*2048); group j streams its own 512
                # W2 columns into PE column group j concurrently (M=32 each)
                scratch = dram.tile([BPC * N, C * R], BF16)

                def reshard(c2):
                    # scratch cols [c2*4096, +4096) are final once sets
                    # 2*c2, 2*c2+1 have written; reshard them immediately so
                    # stage 3 is not gated on the whole of phase A.  Pure DMA:
                    # b2 was already added during the PSUM evacuation.
                    for b in range(BPC):
                        dst = projf[:, b * CH + c2 * N:b * CH + c2 * N + N, :]
                        nc.gpsimd.dma_start(
                            dst,
                            scratch[b * N:(b + 1) * N, c2 * 4096:(c2 + 1) * 4096]
                            .rearrange("n (p r) -> p n r", p=128, r=R))

                for s in range(4):
                    psum = [psW.tile([128, 512], F32, tag="pj", name=f"pj{s}_{j}")
                            for j in range(4)]
                    for ko in range(8):
                        for j in range(4):
                            wch = w2sb[2 * s + j // 2]
                            q2 = j % 2
                            nc.tensor.matmul(
                                psum[j][32 * j:32 * j + 32, :],
                                hT[:, ko, :],
                                wch[:, ko, q2 * 512:(q2 + 1) * 512],
                                start=(ko == 0), stop=(ko == 7),
                                tile_position=(0, 32 * j))
                    pjsb = pAs.tile([128, 512], BF16, tag="pjsb")
                    for j in range(4):
                        # evacuation fused with the b2 bias add (b2 slab in
                        # blobC is grouped/replicated to match [32j+bn, q])
                        nc.vector.tensor_add(
                            pjsb[32 * j:32 * j + 32, :],
                            psum[j][32 * j:32 * j + 32, :],
                            blobC[32 * j:32 * j + 32,
                                  776 + s * 512:776 + (s + 1) * 512])
                        nc.scalar.dma_start(
                            scratch[:, s * 2048 + j * 512:s * 2048 + (j + 1) * 512],
                            pjsb[32 * j:32 * j + 32, :])
                    if s == 1:
                        reshard(0)
                    elif s == 3:
                        reshard(1)

            # ---- phase X: streamed x tiles, col-tiled stage 3, emit -----------
            xgs = {}
            for b in range(BPC):
                for g in range(XT):
                    xg = pXg.tile([128, GCH, T], FP8 if X_FP8 else BF16,
                                  tag="xg", name=f"xg{b}_{g}")
                    nc.sync.dma_start(xg, xq[b, :, g * GCH:(g + 1) * GCH, :])
                    xgs[(b, g)] = xg

            with tc.tile_pool(name="pXw", bufs=6) as pXw, \
                 tc.tile_pool(name="pY", bufs=2) as pY, \
                 tc.tile_pool(name="obuf4", bufs=4) as obuf4, \
                 tc.tile_pool(name="yp", bufs=4, space="PSUM") as yp, \
                 tc.tile_pool(name="op", bufs=4, space="PSUM") as op:

                def s3(b, pys):
                    # stage 3: the four T-quarters of sample b accumulate
                    # across the 32 nc-chunks in four concurrent PE column
                    # groups (M=32 each), one PSUM bank per quarter.  Even
                    # chunks (c-half 0) of each x tile run first so the
                    # c-half-1 reshard can complete in their shadow.
                    seq = [g * GCH + e for g in range(XT)
                           for e in (0, 2, 4, 6, 1, 3, 5, 7)]
                    for i, ch in enumerate(seq):
                        lhs = projf[:, b * CH + (ch % 2) * N + ch // 2, :]
                        xg = xgs[(b, ch // GCH)]
                        for q in range(4):
                            nc.tensor.matmul(
                                pys[q][32 * q:32 * q + 32, :],
                                lhs,
                                xg[:, ch % GCH, q * QT:(q + 1) * QT],
                                start=(i == 0), stop=(i == CH - 1),
                                tile_position=(0, 32 * q))

                def yfin(b, pys):
                    # evacuate quarters (lane-aligned), consolidate into the
                    # contiguous padded ysbc rows [64b, 64b+32), then one
                    # shifted copy per conv tap
                    ysbq = pY.tile([128, QT], BF16, tag="ysbq", name=f"ysbq{b}")
                    for q in range(4):
                        nc.vector.tensor_copy(ysbq[32 * q:32 * q + 32, :],
                                              pys[q][32 * q:32 * q + 32, :])
                    h0 = 64 * b
                    for q in range(4):
                        eng = nc.scalar if q % 2 == 0 else nc.gpsimd
                        eng.dma_start(ysbc[h0:h0 + R, 2 + q * QT:2 + (q + 1) * QT],
                                      ysbq[32 * q:32 * q + 32, :])
                    yk0 = pY.tile([128, T], BF16, tag="yk0", name=f"yk0_{b}")
                    yk1 = pY.tile([128, T], BF16, tag="yk1", name=f"yk1_{b}")
                    for q, (_kb, off) in enumerate(JOFF[0:4]):
                        nc.scalar.dma_start(yk0[q * R:(q + 1) * R, :],
                                            ysbc[h0:h0 + R, off + 2:off + 2 + T])
                    for q, (_kb, off) in enumerate(JOFF[4:8]):
                        nc.gpsimd.dma_start(yk1[q * R:(q + 1) * R, :],
                                            ysbc[h0:h0 + R, off + 2:off + 2 + T])
                    return yk0, yk1

                def emit(b, yk0, yk1):
                    # emit: 3 stacked matmuls (tap 8 + bo ride directly on the
                    # ysbc rows incl. its ones row) + LN + relu per tile; the
                    # gamma/beta/relu passes run pair-merged on two tiles
                    zn2 = None
                    for ts in range(T // 128):
                        po = op.tile([128, C], F32, tag="o")
                        nc.tensor.matmul(po, yk0[:, ts * 128:(ts + 1) * 128],
                                         mst0, start=True, stop=False)
                        nc.tensor.matmul(po, yk1[:, ts * 128:(ts + 1) * 128],
                                         mst1, start=False, stop=False)
                        nc.tensor.matmul(
                            po, ysbc[64 * b:64 * b + R + 1,
                                     4 + ts * 128:4 + (ts + 1) * 128],
                            mcat[64 * b:64 * b + R + 1, :],
                            start=False, stop=True,
                            tile_position=(64 * b, 0))
                        st = pXw.tile([128, 6], F32, tag="st")
                        nc.vector.bn_stats(out=st, in_=po)
                        mv = pXw.tile([128, 2], F32, tag="mv")
                        nc.vector.bn_aggr(out=mv, in_=st)
                        rs = pXw.tile([128, 1], F32, tag="rs")
                        nc.scalar.activation(out=rs, in_=mv[:, 1:2], func=SQRT,
                                             bias=epsb, scale=1.0)
                        nc.vector.reciprocal(rs, rs)
                        nmr = pXw.tile([128, 1], F32, tag="nmr")
                        nc.vector.tensor_scalar(nmr, mv[:, 0:1], rs[:, 0:1], -1.0,
                                                mybir.AluOpType.mult,
                                                mybir.AluOpType.mult)
                        if ts % 2 == 0:
                            zn2 = pXw.tile([128, 2, C], BF16, tag="zn2")
                        nc.scalar.activation(out=zn2[:, ts % 2, :], in_=po,
                                             func=IDENT,
                                             bias=nmr[:, 0:1], scale=rs[:, 0:1])
                        if ts % 4 == 0:
                            ob = obuf4.tile([128, 4, C], BF16, tag="ob4",
                                            name=f"ob{b}_{ts}")
                        if ts % 2 == 1:
                            zg2 = pXw.tile([128, 2, C], BF16, tag="zg2")
                            nc.gpsimd.tensor_mul(zg2, zn2,
                                                 gb16[:, 0:1, :].to_broadcast((128, 2, C)))
                            nc.gpsimd.tensor_add(zg2, zg2,
                                                 bb16[:, 0:1, :].to_broadcast((128, 2, C)))
                            h = (ts % 4) - 1
                            nc.vector.tensor_scalar_max(ob[:, h:h + 2, :], zg2, 0.0)
                        if ts % 4 == 3:
                            nc.scalar.dma_start(
                                out[b, ts // 4, :, :]
                                .rearrange("p (q c) -> p q c", q=4), ob)

                for b in range(BPC):
                    pys = [yp.tile([128, QT], F32, tag="y", name=f"py{b}_{q}")
                           for q in range(4)]
                    s3(b, pys)
                    yk0, yk1 = yfin(b, pys)
                    emit(b, yk0, yk1)

    nc.compile()
    return nc


_NC = None


def _get_nc():
    global _NC
    if _NC is None:
        _NC = _build()
    return _NC


def _shard(inputs):
    """Split full inputs into per-core input maps (layout/cast only)."""
    x = np.asarray(inputs["context_emb"], dtype=np.float32)
    B = x.shape[0]
    assert B == NCORES * BPC
    # [B, T, N, C] -> [B, NCF, T] -> [B, 128, CH, T], fp8 e3m4 at 2x scale
    xT = np.swapaxes(x.reshape(B, T, NCF), 1, 2)
    xT = np.ascontiguousarray(
        np.swapaxes(xT.reshape(B, CH, 128, T), 1, 2))
    if X_FP8:
        xq = np.clip(xT * 2.0, -15.0, 15.0).astype(E3)
        w2scale = 0.5
    else:
        xq = xT.astype(BF)
        w2scale = 1.0
    ph = np.asarray(inputs["phrase_slot"], dtype=np.float32)
    w2 = np.asarray(inputs["W2"], dtype=np.float32) * w2scale
    w2h = np.ascontiguousarray(
        w2.reshape(8, 128, 8, 1024).transpose(1, 2, 0, 3)).astype(BF)
    w1 = np.asarray(inputs["W1"], dtype=np.float32)
    w1h = w1.reshape(4, 128, 4 * C).transpose(1, 0, 2).reshape(128, 4096)
    kjoin = np.concatenate(
        [np.moveaxis(inputs[f"k{k}"], 2, 0) for k in (1, 3, 5)],
        axis=0)  # [9, 32, 256]
    kjh = np.moveaxis(kjoin, 1, 0).reshape(R, 9 * C)  # [r, (j d)]
    wo = np.asarray(inputs["Wo"], dtype=np.float32)
    woh = wo.reshape(6, 128, C).transpose(1, 0, 2).reshape(128, 6 * C)
    eos = np.asarray(inputs["eos_slot"], dtype=np.float32).reshape(2, 128).T
    blobA = np.ascontiguousarray(
        np.concatenate([w1h, woh, eos], axis=1)).astype(BF)
    b1c = np.asarray(inputs["b1"], dtype=np.float32).reshape(8, 128).T
    lnp = np.concatenate([
        np.asarray(inputs["gamma"], dtype=np.float32),
        np.asarray(inputs["beta"], dtype=np.float32),
        np.asarray(inputs["bo"], dtype=np.float32)])
    # b2 slab grouped per (set, col-group): row (j*32+bn), col (s, q) holds
    # b2[s*2048 + j*512 + q], replicated across the 32 bn rows of group j
    b2s = np.asarray(inputs["b2"], dtype=np.float32) * w2scale
    b2r = np.broadcast_to(
        b2s.reshape(4, 4, 512).transpose(1, 0, 2)[:, None, :, :],
        (4, 32, 4, 512)).reshape(128, 2048)
    blobC = np.ascontiguousarray(np.concatenate(
        [b1c, np.broadcast_to(lnp, (128, 3 * C)), b2r], axis=1),
        dtype=np.float32)
    shared = {
        "blobA": blobA,
        "blobC": blobC,
        "w2h": w2h,
    }
    in_maps = []
    for i in range(NCORES):
        m = dict(shared)
        m["xq"] = np.ascontiguousarray(xq[i * BPC:(i + 1) * BPC])
        m["blobB"] = np.ascontiguousarray(np.concatenate(
            [ph[i * BPC:(i + 1) * BPC].reshape(BPC * N, C), kjh],
            axis=1)).astype(BF)
        in_maps.append(m)
    return in_maps


def _run(inputs, **kwargs):
    nc = _get_nc()
    res = run_bass_kernel_spmd(nc, _shard(inputs), core_ids=list(range(NCORES)),
                               **kwargs)
    outs = [r["out"] for r in res.results]
    full = np.concatenate(outs, axis=0).reshape(NCORES * BPC, 2, 128, 4, C)
    # [b, s, p, q, c] -> t = (s*4 + q)*128 + p
    full = np.ascontiguousarray(full.transpose(0, 1, 3, 2, 4)).reshape(
        NCORES * BPC, T, C)
    return full.astype(np.float32), res


def kernel(**inputs) -> np.ndarray:
    out, _ = _run(inputs)
    return out
